# revision 1
# baseline (speedup 1.0000x reference)
"""Trainium2 kernel for nn_GroupedStackedAFDF.

Every op in the reference (block-diagonal complex matmul, FFT, IFFT, channel
permutation) is linear along the channel axis with fixed weights, so the whole
4-layer network collapses into a single complex matrix T with
    out = Re(T @ z) = Re(T) @ x          (x is real)
T is built on host from the tiny weights (exact, complex128); the device then
runs one dense [32768,1024] @ [1024,1024] real matmul, data-parallel over the
batch dim across 8 cores (4096 rows/core).

Device layout: everything is computed transposed (channels on partitions) so
both DMA directions are fully contiguous:
    outT[ch_out, b] = W.T @ xT   with  W = Re(T).T  ([ch_in, ch_out])
lhsT tiles are W[k*128:.., m*128:..] ([K,M]), rhs tiles are xT[k*128:.., bchunk]
([K,N]); bf16 operands (full PE rate + FWL), fp32 PSUM accumulate/output.
"""

import numpy as np
import ml_dtypes

import concourse.bass as bass
from concourse import bacc
import concourse.mybir as mybir
from concourse.tile import TileContext
from concourse.bass_utils import run_bass_kernel_spmd

N, D, L, G = 32768, 1024, 4, 32
DG = D // G
NCORES = 8
NB = N // NCORES          # 4096 batch rows per core
BCH = 512                 # batch chunk = psum free dim
NKT = D // 128            # 8 contraction tiles
NMT = D // 128            # 8 output-channel tiles
NCH = NB // BCH           # 8 batch chunks per core

_BF16 = mybir.dt.bfloat16
_F32 = mybir.dt.float32
WARMUP_MMS = 8


def _build_T(Aa, Ab, Da, Db, perms):
    """Compose the network into one complex [D, D] matrix acting on channel
    vectors: z_out = T @ z_in."""
    T = np.eye(D, dtype=np.complex128)
    for l in range(L):
        Wa = Aa[l].astype(np.float64) + 1j * Ab[l].astype(np.float64)
        Wd = Da[l].astype(np.float64) + 1j * Db[l].astype(np.float64)
        T = np.einsum("gok,gkc->goc", Wa, T.reshape(G, DG, D)).reshape(D, D)
        T = np.fft.fft(T, axis=0)
        T = np.einsum("gok,gkc->goc", Wd, T.reshape(G, DG, D)).reshape(D, D)
        T = np.fft.ifft(T, axis=0)
        T = T[np.asarray(perms[l]), :]
    return T


def _build_nc():
    nc = bacc.Bacc("TRN2", target_bir_lowering=False, enable_partition_id=False)
    xT = nc.declare_dram_parameter("xT", [D, NB], _BF16, isOutput=False)
    W = nc.declare_dram_parameter("W", [D, D], _BF16, isOutput=False)
    outT = nc.declare_dram_parameter("outT", [D, NB], _F32, isOutput=True)

    with TileContext(nc) as tc:
        with (
            tc.tile_pool(name="wpool", bufs=1) as wpool,
            tc.tile_pool(name="xpool", bufs=1) as xpool,
            tc.tile_pool(name="pspool", bufs=7, space="PSUM") as pspool,
            tc.tile_pool(name="opool", bufs=4) as opool,
        ):
            # PE warm-up: dummy matmuls on a zeroed tile keep the PE busy
            # while the first loads are in flight, so HAM is at 8/8 when the
            # real matmuls start.
            warm_x = wpool.tile([128, BCH], _BF16, tag="warmx", name="warm_x")
            nc.vector.memset(warm_x[:], 0.0)
            warm_ps = pspool.tile([128, BCH], _F32, tag="warm", bufs=1, name="warm_ps")
            for _ in range(WARMUP_MMS):
                nc.tensor.matmul(
                    warm_ps[:], warm_x[:, 0:128], warm_x[:], start=True, stop=True
                )

            # W is pre-arranged on host so row-block m holds all 8 [128,128]
            # lhsT blocks for output-channel tile m side by side:
            #   W[m*128+p, k*128+q] = Wmat[k*128+p, m*128+q]
            # DMA *issue* on a sequencer costs ~650 ns each, so instruction
            # order and queue placement below are chosen to get the first
            # matmul group's data (wm0 + first 512-col slice of x) in flight
            # as early as possible, split across both HWDGE sequencers.
            wt = [None] * NMT
            w_tile = wpool.tile([128, D], _BF16, tag="w0", name="w0")
            nc.sync.dma_start(out=w_tile[:], in_=W[0:128, :])
            wt[0] = w_tile

            # chunk-0 x loads split across both HWDGE sequencers for the
            # fastest possible start; later chunks all go on sync while
            # scalar handles the output stores.
            xt0 = []
            for k in range(NKT):
                x_tile = xpool.tile([128, BCH], _BF16, tag=f"x{k}", bufs=2, name=f"x{k}_0")
                eng = nc.sync if k % 2 == 0 else nc.scalar
                eng.dma_start(
                    out=x_tile[:], in_=xT[k * 128 : (k + 1) * 128, 0:BCH]
                )
                xt0.append(x_tile)
            for m in range(1, NMT):
                w_tile = wpool.tile([128, D], _BF16, tag=f"w{m}", name=f"w{m}")
                nc.sync.dma_start(out=w_tile[:], in_=W[m * 128 : (m + 1) * 128, :])
                wt[m] = w_tile

            xt = xt0
            for b in range(NCH):
                bsl = slice(b * BCH, (b + 1) * BCH)
                if b > 0:
                    xt = []
                    for k in range(NKT):
                        x_tile = xpool.tile(
                            [128, BCH], _BF16, tag=f"x{k}", bufs=2, name=f"x{k}_{b}"
                        )
                        nc.sync.dma_start(
                            out=x_tile[:], in_=xT[k * 128 : (k + 1) * 128, bsl]
                        )
                        xt.append(x_tile)
                for m in range(NMT):
                    ps = pspool.tile([128, BCH], _F32, tag="ps", name=f"ps{b}_{m}")
                    msl = slice(m * 128, (m + 1) * 128)
                    for k in range(NKT):
                        nc.tensor.matmul(
                            ps[:],
                            wt[m][:, k * 128 : (k + 1) * 128],
                            xt[k][:],
                            start=(k == 0),
                            stop=(k == NKT - 1),
                        )
                    o_tile = opool.tile([128, BCH], _F32, tag="o", name=f"o{b}_{m}")
                    nc.vector.tensor_copy(o_tile[:], ps[:])
                    nc.scalar.dma_start(out=outT[msl, bsl], in_=o_tile[:])
    nc.finalize()
    return nc


_nc_cache = {}


def _get_nc():
    if "nc" not in _nc_cache:
        _nc_cache["nc"] = _build_nc()
    return _nc_cache["nc"]


def _run_device(xT_bf16, W_bf16, trace=False, **kw):
    """xT_bf16: [D, N] bf16, W_bf16: [D, D] bf16. Returns (out [N, D] f32, result)."""
    nc = _get_nc()
    in_maps = [
        {
            "xT": np.ascontiguousarray(xT_bf16[:, c * NB : (c + 1) * NB]),
            "W": W_bf16,
        }
        for c in range(NCORES)
    ]
    try:
        res = run_bass_kernel_spmd(nc, in_maps, list(range(NCORES)), trace=trace, **kw)
    except Exception:
        # transient NRT/device hiccups have been observed; retry once
        res = run_bass_kernel_spmd(nc, in_maps, list(range(NCORES)), trace=trace, **kw)
    out = np.empty((N, D), np.float32)
    for c in range(NCORES):
        out[c * NB : (c + 1) * NB, :] = res.results[c]["outT"].T
    return out, res


def _prep_W(T):
    """bf16 weights, rearranged m-major: W[m*128+p, k*128+q] = Re(T).T[k*128+p, m*128+q]."""
    Wmat = np.real(T).T.astype(ml_dtypes.bfloat16)       # [ch_in, ch_out]
    return np.ascontiguousarray(
        Wmat.reshape(NKT, 128, NMT, 128).transpose(2, 1, 0, 3).reshape(D, D)
    )


def kernel(x, Aa, Ab, Da, Db, perms):
    x = np.asarray(x, dtype=np.float32)
    Aa, Ab, Da, Db = (np.asarray(a, dtype=np.float32) for a in (Aa, Ab, Da, Db))
    perms = np.asarray(perms)
    assert x.shape == (N, D), x.shape
    T = _build_T(Aa, Ab, Da, Db, perms)
    W = _prep_W(T)
    xT = np.ascontiguousarray(x.T).astype(ml_dtypes.bfloat16)  # [D, N]
    out, _ = _run_device(xT, W, trace=False)
    return out

